# revision 23
# baseline (speedup 1.0000x reference)
"""Trainium2 Bass kernel for nn_HBClassicNet.

Net: fc1 -> BN1(+ReLU) -> poincare log-map -> 3-stage butterfly -> exp-map
     -> BN2(+ReLU) -> fc2

Key algebraic simplifications (host-side, batch-independent only):
  * The 3 butterfly stages compose into one 256x256 block-diagonal matrix B
    (two independent 128x128 blocks -> only those blocks are shipped).
  * The log/exp-map scales are per-row scalars and commute with B.
  * fc1 bias cancels exactly in BN1 (affine followed by batch-norm).

Sharding: pure data-parallel over the batch (32768 rows -> 8 x 4096).
BN batch statistics are made global with two tiny AllGathers (one ring
sweep instead of AllReduce's two) + a local 3-op tree sum.  A dummy
AllGather issued first-thing absorbs the multi-10us core launch stagger
and CC firmware warmup concurrently with the x load and fc1, so the real
stats collectives only pay ~1us trigger skew + ~6-13us exec.

Performance notes (from perfetto/NTFF analysis):
  * PE runs ~1.2-1.45GHz here; fc1 (57K cols) / fc2 (64K cols) dominate.
  * bf16 everywhere the ~2e-2 error budget allows.  fp8 is NOT usable:
    fc2 has ~250x output cancellation (max|out| ~ 0.004 from O(1) terms).
  * Inputs are repacked on the host so every DMA reads contiguous
    per-partition lines (6KB), and descriptors are spread across the two
    hardware DGE queues (sync/scalar).  gpsimd.dma_start (software DGE)
    is avoided for large patterns - the Q7 descriptor generation is slow.
  * Row norms accumulate into one [2, RCW] PSUM tile (matmul with
    [ones|0] / [0|ones] stationary vectors, interleaved accumulation
    groups); a single copy moves both norms out.
  * The T-coefficient chain is computed per half-shard with ACT ops
    grouped by function (Sqrt,Sqrt,Ln,Ln,Ln,Ln,Tanh,Tanh) - each ACT
    table switch costs 1.28us.
  * T is broadcast across partitions by gpsimd.partition_broadcast from
    a dedicated tile pool (sharing a pool with DVE-recycled tiles
    serializes the Pool queue against DVE).
  * BN2+fc2 are interleaved per row-chunk; output is written bf16 and
    upcast to f32 on the host.
"""

import numpy as np

B_FULL, IN_DIM, HID, OUT_DIM = 32768, 784, 256, 1000
NCORES = 8
RS = B_FULL // NCORES  # 4096 rows per shard
L, CURV = 3, 1e-3
LOG2_H = 8
EPS_BN = 1e-5

RC = 8          # row chunks per shard
RCW = RS // RC  # 512 rows per chunk
KC6 = 6         # full 128-partition K chunks of IN_DIM
KREM = IN_DIM - KC6 * 128  # 16

RECOMPUTE_FC1 = False

_cache = {}


def _butterfly_matrix(params):
    """Compose the L butterfly stages into one dense [HID, HID] matrix (f64)."""
    p64 = np.asarray(params, dtype=np.float64)
    Bm = np.eye(HID, dtype=np.float64)
    off = 0
    for l in range(L):
        bs = 1 << (l % LOG2_H)
        nb = HID // (2 * bs)
        a = p64[off:off + nb]
        b = p64[off + nb:off + 2 * nb]
        S = np.zeros((HID, HID), dtype=np.float64)
        for blk in range(nb):
            base = blk * 2 * bs
            i1 = np.arange(base, base + bs)
            i2 = i1 + bs
            S[i1, i1] = a[blk]
            S[i1, i2] = b[blk]
            S[i2, i1] = -b[blk]
            S[i2, i2] = a[blk]
        Bm = S @ Bm
        off += 2 * nb
    return Bm


def _build(has_bias, recompute=RECOMPUTE_FC1):
    import concourse.bacc as bacc
    import concourse.tile as tile
    import concourse.mybir as mybir

    f32 = mybir.dt.float32
    bf16 = mybir.dt.bfloat16
    AF = mybir.ActivationFunctionType
    ALU = mybir.AluOpType

    nc = bacc.Bacc(
        "TRN2",
        target_bir_lowering=False,
        debug=False,
        enable_asserts=False,
        num_devices=NCORES,
    )

    # p-major packed layouts: per-partition data is contiguous in DRAM so
    # DMA packets are large (6KB lines) instead of 1KB strided
    xT_d = nc.dram_tensor("xT", [RC, 128, KC6 * RCW], bf16, kind="ExternalInput")
    xtl_d = nc.dram_tensor("xtl", [KREM, RC * RCW], bf16, kind="ExternalInput")
    w1tl_d = nc.dram_tensor("w1tl", [KREM, HID], bf16, kind="ExternalInput")
    w1T_d = nc.dram_tensor("w1T", [128, KC6 * HID], bf16, kind="ExternalInput")
    bT_d = nc.dram_tensor("bT", [128, 2 * 128], bf16, kind="ExternalInput")
    w2T_d = nc.dram_tensor("w2T", [128, 2 * OUT_DIM], bf16, kind="ExternalInput")
    smalls_d = nc.dram_tensor("smalls", [8, 128], f32, kind="ExternalInput")
    if has_bias:
        b2_d = nc.dram_tensor("b2row", [1, OUT_DIM], f32, kind="ExternalInput")
    out_d = nc.dram_tensor("out", [RS, OUT_DIM], bf16, kind="ExternalOutput")

    with tile.TileContext(nc) as tc:
        with (
            tc.tile_pool(name="const", bufs=1) as constp,
            tc.tile_pool(name="big", bufs=1) as bigp,
            tc.tile_pool(name="row", bufs=1) as rowp,
            tc.tile_pool(name="small", bufs=1) as smallp,
            tc.tile_pool(name="cmp", bufs=16) as cmpp,
            tc.tile_pool(name="sqt", bufs=6) as sqtp,
            tc.tile_pool(name="tbp", bufs=3) as tbp,
            tc.tile_pool(name="outp", bufs=3) as outp,
            tc.tile_pool(name="psmm", bufs=6, space="PSUM") as psmm,
            tc.tile_pool(name="psnorm", bufs=2, space="PSUM") as psnorm,
            tc.tile_pool(name="dram", bufs=1, space="DRAM") as dramp,
        ):
            # warmup collective first: absorbs core launch stagger + CC
            # firmware warmup concurrently with the x load and fc1
            with nc.named_scope("warmup"):
                wuin = dramp.tile([128, 1], f32, tag="wuin")
                wuout = dramp.tile([NCORES * 128, 1], f32, tag="wuout")
                wusb = smallp.tile([128, 1], f32, tag="wusb")
                nc.vector.memset(wusb[:, :], 0.0)
                nc.scalar.dma_start(out=wuin[:, :], in_=wusb[:, :])
                nc.gpsimd.collective_compute(
                    "AllGather",
                    ALU.bypass,
                    replica_groups=[list(range(NCORES))],
                    ins=[wuin.opt()],
                    outs=[wuout.opt()],
                )

            # ---------------- constants (weights first, then x chunk 0) ------
            w1t6 = constp.tile([128, KC6, HID], bf16, tag="w1t6")
            nc.sync.dma_start(
                out=w1t6[:, :, :],
                in_=w1T_d[:, :].rearrange("p (k m) -> p k m", k=KC6),
            )
            w1t1 = constp.tile([KREM, HID], bf16, tag="w1t1")
            nc.scalar.dma_start(out=w1t1[:, :], in_=w1tl_d[:, :])

            xall = bigp.tile([128, RC, KC6, RCW], bf16, tag="xall")
            xtail = bigp.tile([KREM, RC, RCW], bf16, tag="xtail")

            def xload(rc):
                engs = [nc.sync, nc.scalar]
                if rc == 0:
                    # k=0 alone first: the first fc1 matmul only waits ~130KB
                    nc.sync.dma_start(
                        out=xall[:, 0, 0:1, :],
                        in_=xT_d[0, :, 0:RCW].rearrange("p (k m) -> p k m", k=1),
                    )
                    nc.scalar.dma_start(
                        out=xall[:, 0, 1:3, :],
                        in_=xT_d[0, :, RCW:3 * RCW].rearrange(
                            "p (k m) -> p k m", k=2
                        ),
                    )
                    nc.sync.dma_start(
                        out=xall[:, 0, 3:6, :],
                        in_=xT_d[0, :, 3 * RCW:6 * RCW].rearrange(
                            "p (k m) -> p k m", k=3
                        ),
                    )
                    nc.scalar.dma_start(
                        out=xtail[:, :, :],
                        in_=xtl_d[:, :].rearrange("p (r m) -> p r m", r=RC),
                    )
                    return
                KH = KC6 // 2
                for h in range(2):
                    ks = slice(h * KH * RCW, (h + 1) * KH * RCW)
                    engs[h % 2].dma_start(
                        out=xall[:, rc, h * KH:(h + 1) * KH, :],
                        in_=xT_d[rc, :, ks].rearrange("p (k m) -> p k m", k=KH),
                    )

            with nc.named_scope("xload"):
                xload(0)

            bt_sb = constp.tile([128, 2, 128], bf16, tag="bt")
            nc.gpsimd.dma_start(
                out=bt_sb[:, :, :],
                in_=bT_d[:, :].rearrange("p (c m) -> p c m", c=2),
            )
            w2t_sb = constp.tile([128, 2, OUT_DIM], bf16, tag="w2t")
            nc.gpsimd.dma_start(
                out=w2t_sb[:, :, :],
                in_=w2T_d[:, :].rearrange("p (k m) -> p k m", k=2),
            )
            smalls = constp.tile([128, 8], f32, tag="smalls")
            nc.gpsimd.dma_start(
                out=smalls[:, :], in_=smalls_d[:, :].rearrange("c p -> p c")
            )
            if has_bias:
                b2row = constp.tile([1, OUT_DIM], bf16, tag="b2row")
                b2f = constp.tile([1, OUT_DIM], f32, tag="b2f")
                nc.gpsimd.dma_start(out=b2f[:, :], in_=b2_d[:, :])
                nc.scalar.copy(b2row[:, :], b2f[:, :])

            ones_k = constp.tile([128, 1], bf16, tag="ones_k")
            nc.vector.memset(ones_k[:, :], 1.0)
            nsel = constp.tile([128, 2, 2], bf16, tag="nsel")
            nc.vector.memset(nsel[:, :, :], 0.0)
            nc.vector.memset(nsel[:, 0, 0:1], 1.0)
            nc.vector.memset(nsel[:, 1, 1:2], 1.0)
            ones_m = constp.tile([1, 128], bf16, tag="ones_m")
            nc.vector.memset(ones_m[:, :], 1.0)
            eps_t = constp.tile([128, 1], f32, tag="eps_t")
            nc.vector.memset(eps_t[:, :], float(EPS_BN))

            with nc.named_scope("xload"):
                for rc in range(1, RC):
                    xload(rc)

            # ---------------- fc1 ----------------
            stat1 = smallp.tile([128, 2, RC, 6], f32, tag="stat1")
            if not recompute:
                h = [
                    bigp.tile([128, RS], bf16, tag=f"h{m}", name=f"h{m}")
                    for m in range(2)
                ]

            def fc1_chunk(rc, mc):
                ms = slice(mc * 128, (mc + 1) * 128)
                ph = psmm.tile([128, RCW], f32, tag="psmm", name="ph")
                for k in range(KC6):
                    nc.tensor.matmul(
                        ph[:, :],
                        w1t6[:, k, ms],
                        xall[:, rc, k, :],
                        start=(k == 0),
                        stop=False,
                    )
                nc.tensor.matmul(
                    ph[:, :], w1t1[:, ms], xtail[:, rc, :],
                    start=False, stop=True,
                )
                return ph

            with nc.named_scope("fc1"):
                for rc in range(RC):
                    cs = slice(rc * RCW, (rc + 1) * RCW)
                    for mc in range(2):
                        ph = fc1_chunk(rc, mc)
                        nc.vector.bn_stats(stat1[:, mc, rc, :], ph[:, :])
                        if not recompute:
                            if (rc + mc) % 2 == 0:
                                nc.scalar.copy(h[mc][:, cs], ph[:, :])
                            else:
                                nc.vector.tensor_copy(h[mc][:, cs], ph[:, :])

            # ---------------- stats AllGather + local sum ----------------
            def stats_allgather(stat, tag):
                """stat [128,2,RC,6] -> allr [128, 4]: global (sum mean, sum E[x^2])."""
                aggr = smallp.tile([128, 2, 2], f32, tag=f"aggr{tag}", name=f"aggr{tag}")
                pay = smallp.tile([128, 4], f32, tag=f"pay{tag}", name=f"pay{tag}")
                for mc in range(2):
                    nc.vector.bn_aggr(aggr[:, mc, :], stat[:, mc, :, :])
                msq = cmpp.tile([128, 2], f32, tag="cmp", name=f"msq{tag}")
                nc.vector.tensor_mul(msq[:, :], aggr[:, :, 0], aggr[:, :, 0])
                nc.vector.tensor_copy(pay[:, 0:2], aggr[:, :, 0])
                nc.vector.tensor_add(pay[:, 2:4], aggr[:, :, 1], msq[:, :])
                arin = dramp.tile([128, 4], f32, tag=f"arin{tag}", name=f"arin{tag}")
                arout = dramp.tile(
                    [NCORES * 128, 4], f32, tag=f"arout{tag}", name=f"arout{tag}"
                )
                nc.sync.dma_start(out=arin[:, :], in_=pay[:, :])
                nc.gpsimd.collective_compute(
                    "AllGather",
                    ALU.bypass,
                    replica_groups=[list(range(NCORES))],
                    ins=[arin.opt()],
                    outs=[arout.opt()],
                )
                allg = smallp.tile(
                    [128, NCORES, 4], f32, tag=f"allg{tag}", name=f"allg{tag}"
                )
                nc.sync.dma_start(
                    out=allg[:, :, :],
                    in_=arout[:, :].rearrange("(r p) c -> p r c", p=128),
                )
                s4 = cmpp.tile([128, 4, 4], f32, tag="cmp", name=f"s4{tag}")
                nc.vector.tensor_add(s4[:, :, :], allg[:, 0:4, :], allg[:, 4:8, :])
                s2 = cmpp.tile([128, 2, 4], f32, tag="cmp", name=f"s2{tag}")
                nc.vector.tensor_add(s2[:, :, :], s4[:, 0:2, :], s4[:, 2:4, :])
                allr = smallp.tile([128, 1, 4], f32, tag=f"allr{tag}", name=f"allr{tag}")
                nc.vector.tensor_add(allr[:, :, :], s2[:, 0:1, :], s2[:, 1:2, :])
                return allr

            def bn_scale_bias(allr, gcol, bcol, tag):
                """Global stats -> per-partition scale/bias tiles [128, 2]."""
                scale = smallp.tile([128, 2], f32, tag=f"scale{tag}", name=f"scale{tag}")
                bias = smallp.tile([128, 2], f32, tag=f"bias{tag}", name=f"bias{tag}")
                mean = cmpp.tile([128, 2], f32, tag="cmp", name=f"mean{tag}")
                nc.vector.tensor_scalar_mul(mean[:, :], allr[:, 0, 0:2], 1.0 / NCORES)
                m2 = cmpp.tile([128, 2], f32, tag="cmp", name=f"m2{tag}")
                nc.vector.tensor_mul(m2[:, :], mean[:, :], mean[:, :])
                var = cmpp.tile([128, 2], f32, tag="cmp", name=f"var{tag}")
                nc.vector.scalar_tensor_tensor(
                    out=var[:, :], in0=allr[:, 0, 2:4], scalar=1.0 / NCORES,
                    in1=m2[:, :], op0=ALU.mult, op1=ALU.subtract,
                )
                std = cmpp.tile([128, 2], f32, tag="cmp", name=f"std{tag}")
                nc.scalar.activation(std[:, :], var[:, :], AF.Sqrt, bias=eps_t[:, :])
                rstd = cmpp.tile([128, 2], f32, tag="cmp", name=f"rstd{tag}")
                nc.vector.reciprocal(rstd[:, :], std[:, :])
                nc.vector.tensor_mul(scale[:, :], rstd[:, :], smalls[:, gcol:gcol + 2])
                mneg = cmpp.tile([128, 2], f32, tag="cmp", name=f"mneg{tag}")
                nc.vector.scalar_tensor_tensor(
                    out=mneg[:, :], in0=allr[:, 0, 0:2], scalar=-1.0 / NCORES,
                    in1=scale[:, :], op0=ALU.mult, op1=ALU.mult,
                )
                nc.vector.tensor_add(bias[:, :], mneg[:, :], smalls[:, bcol:bcol + 2])
                return scale, bias

            with nc.named_scope("ar1"):
                allr1 = stats_allgather(stat1, "1")
                scale1, bias1 = bn_scale_bias(allr1, 0, 2, "1")

            # ---------------- middle: BN1+ReLU, butterfly, norms, T ----------
            hbn = [
                bigp.tile([128, RS], bf16, tag=f"hbn{m}", name=f"hbn{m}")
                for m in range(2)
            ]
            w = [
                bigp.tile([128, RS], bf16, tag=f"w{m}", name=f"w{m}")
                for m in range(2)
            ]
            ht = [
                bigp.tile([128, RS], bf16, tag=f"ht{m}", name=f"ht{m}")
                for m in range(2)
            ]
            trow = rowp.tile([1, RS], bf16, tag="trow")
            stat2 = smallp.tile([128, 2, RC, 6], f32, tag="stat2")
            nrow2 = rowp.tile([2, RS], f32, tag="nrow2")

            nhalf = RC // 2
            HW = nhalf * RCW  # 2048 rows per half

            def mid_chunk(rc):
                cs = slice(rc * RCW, (rc + 1) * RCW)
                with nc.named_scope("bn1"):
                    for mc in range(2):
                        if recompute:
                            src_ = fc1_chunk(rc, mc)[:, :]
                        else:
                            src_ = h[mc][:, cs]
                        nc.scalar.activation(
                            hbn[mc][:, cs], src_, AF.Relu,
                            bias=bias1[:, mc:mc + 1], scale=scale1[:, mc:mc + 1],
                        )
                with nc.named_scope("norms"):
                    pn = psnorm.tile([2, RCW], f32, tag="psn", name="pn")
                    sqs = []
                    for mc in range(2):
                        sqt = sqtp.tile([128, RCW], bf16, tag="sqt", name="sqt")
                        nc.vector.tensor_mul(sqt[:, :], hbn[mc][:, cs], hbn[mc][:, cs])
                        sqs.append(sqt)
                    nc.tensor.matmul(
                        pn[:, :], nsel[:, 0, :], sqs[0][:, :],
                        start=True, stop=False, skip_group_check=True,
                    )
                    nc.tensor.matmul(
                        pn[:, :], nsel[:, 0, :], sqs[1][:, :],
                        start=False, stop=False, skip_group_check=True,
                    )
                    for mc in range(2):
                        pw = psmm.tile([128, RCW], f32, tag="psmm", name="pw")
                        nc.tensor.matmul(
                            pw[:, :], bt_sb[:, mc, :], hbn[mc][:, cs],
                            start=True, stop=True,
                        )
                        if mc == 0:
                            nc.scalar.copy(w[mc][:, cs], pw[:, :])
                        else:
                            nc.vector.tensor_copy(w[mc][:, cs], pw[:, :])
                        sqwt = sqtp.tile([128, RCW], bf16, tag="sqt", name="sqwt")
                        nc.vector.tensor_mul(sqwt[:, :], w[mc][:, cs], w[mc][:, cs])
                        nc.tensor.matmul(
                            pn[:, :], nsel[:, 1, :], sqwt[:, :],
                            start=False, stop=(mc == 1), skip_group_check=True,
                        )
                    nc.scalar.copy(nrow2[:, cs], pn[:, :])

            def rowscalars_pair():
                C = HW // 128  # 16 rows-per-partition per half
                ncc, sq2, sn1, snwf, la, lb = [], [], [], [], [], []
                at2, r1, m1, m2t, th, rw, tco = [], [], [], [], [], [], []
                for hf in range(2):
                    hs = slice(hf * HW, (hf + 1) * HW)
                    t = cmpp.tile([128, 2, C], f32, tag="cmp", name=f"ncc{hf}")
                    nc.scalar.dma_start(
                        out=t[:, 0, :],
                        in_=nrow2[0:1, hs].rearrange("o (a b) -> o a b", a=128),
                    )
                    nc.scalar.dma_start(
                        out=t[:, 1, :],
                        in_=nrow2[1:2, hs].rearrange("o (a b) -> o a b", a=128),
                    )
                    ncc.append(t)
                for hf in range(2):
                    t = cmpp.tile([128, 2, C], f32, tag="cmp", name=f"sq2{hf}")
                    nc.scalar.activation(
                        t[:, :, :], ncc[hf][:, :, :], AF.Sqrt, scale=float(CURV)
                    )
                    sq2.append(t)
                for hf in range(2):
                    t = cmpp.tile([128, C], f32, tag="cmp", name=f"sn1{hf}")
                    nc.vector.tensor_scalar(
                        out=t[:, :], in0=sq2[hf][:, 0, :],
                        scalar1=1.0 - 1e-6, scalar2=1e-7,
                        op0=ALU.min, op1=ALU.max,
                    )
                    sn1.append(t)
                    t = cmpp.tile([128, C], f32, tag="cmp", name=f"snwf{hf}")
                    nc.vector.tensor_scalar(
                        out=t[:, :], in0=sq2[hf][:, 1, :],
                        scalar1=1e-20, scalar2=None, op0=ALU.max,
                    )
                    snwf.append(t)
                for hf in range(2):
                    t = cmpp.tile([128, C], f32, tag="cmp", name=f"la{hf}")
                    nc.scalar.activation(t[:, :], sn1[hf][:, :], AF.Ln, bias=1.0, scale=1.0)
                    la.append(t)
                    t = cmpp.tile([128, C], f32, tag="cmp", name=f"lb{hf}")
                    nc.scalar.activation(t[:, :], sn1[hf][:, :], AF.Ln, bias=1.0, scale=-1.0)
                    lb.append(t)
                for hf in range(2):
                    t = cmpp.tile([128, C], f32, tag="cmp", name=f"at2{hf}")
                    nc.vector.tensor_sub(t[:, :], la[hf][:, :], lb[hf][:, :])
                    at2.append(t)
                    t = cmpp.tile([128, C], f32, tag="cmp", name=f"r1{hf}")
                    nc.vector.reciprocal(t[:, :], sn1[hf][:, :])
                    r1.append(t)
                for hf in range(2):
                    t = cmpp.tile([128, C], f32, tag="cmp", name=f"m1{hf}")
                    nc.vector.tensor_mul(t[:, :], at2[hf][:, :], r1[hf][:, :])
                    m1.append(t)
                for hf in range(2):
                    t = cmpp.tile([128, C], f32, tag="cmp", name=f"m2t{hf}")
                    nc.vector.tensor_mul(t[:, :], m1[hf][:, :], snwf[hf][:, :])
                    m2t.append(t)
                for hf in range(2):
                    t = cmpp.tile([128, C], f32, tag="cmp", name=f"th{hf}")
                    nc.scalar.activation(t[:, :], m2t[hf][:, :], AF.Tanh, scale=0.5)
                    th.append(t)
                for hf in range(2):
                    t = cmpp.tile([128, C], f32, tag="cmp", name=f"rw{hf}")
                    nc.vector.reciprocal(t[:, :], snwf[hf][:, :])
                    rw.append(t)
                for hf in range(2):
                    hs = slice(hf * HW, (hf + 1) * HW)
                    t = cmpp.tile([128, C], bf16, tag="cmp", name=f"tco{hf}")
                    nc.vector.tensor_mul(t[:, :], th[hf][:, :], rw[hf][:, :])
                    nc.scalar.dma_start(
                        out=trow[0:1, hs].rearrange("o (a b) -> o a b", a=128),
                        in_=t[:, :],
                    )

            def applyT_chunk(rc):
                cs = slice(rc * RCW, (rc + 1) * RCW)
                tb = tbp.tile([128, RCW], bf16, tag="tb", name="tb")
                nc.gpsimd.partition_broadcast(tb[:, :], trow[0:1, cs])
                for mc in range(2):
                    nc.vector.tensor_mul(ht[mc][:, cs], w[mc][:, cs], tb[:, :])
                    nc.vector.bn_stats(stat2[:, mc, rc, :], ht[mc][:, cs])

            with nc.named_scope("middle"):
                for rc in range(RC):
                    mid_chunk(rc)
                rowscalars_pair()
                for rc in range(RC):
                    applyT_chunk(rc)

            with nc.named_scope("ar2"):
                allr2 = stats_allgather(stat2, "2")
                scale2, bias2 = bn_scale_bias(allr2, 4, 6, "2")

            # ---------------- BN2 + fc2 interleaved + store ----------------
            ht2 = [
                bigp.tile([128, RS], bf16, tag=f"ht2{m}", name=f"ht2{m}")
                for m in range(2)
            ]
            NH = OUT_DIM // 2  # 500
            with nc.named_scope("fc2"):
                for mp in range(RS // 512):  # 8 quad-row-chunks
                    rc = mp
                    cs = slice(rc * RCW, (rc + 1) * RCW)
                    with nc.named_scope("bn2"):
                        for mc in range(2):
                            nc.scalar.activation(
                                ht2[mc][:, cs], ht[mc][:, cs], AF.Relu,
                                bias=bias2[:, mc:mc + 1],
                                scale=scale2[:, mc:mc + 1],
                            )
                    osb = outp.tile([128, 4, OUT_DIM], bf16, tag="osb", name="osb")
                    for c in range(4):
                        m = 4 * mp + c
                        rs_ = slice(m * 128, (m + 1) * 128)
                        for nch in range(2):
                            ns = slice(nch * NH, (nch + 1) * NH)
                            po = psmm.tile([128, NH], f32, tag="psmm", name="po")
                            nc.tensor.matmul(
                                po[:, :], ht2[0][:, rs_], w2t_sb[:, 0, ns],
                                start=True, stop=False,
                            )
                            nc.tensor.matmul(
                                po[:, :], ht2[1][:, rs_], w2t_sb[:, 1, ns],
                                start=False, stop=(not has_bias),
                            )
                            if has_bias:
                                nc.tensor.matmul(
                                    po[:, :], ones_m[:, :], b2row[0:1, ns],
                                    start=False, stop=True,
                                )
                            if (m + nch) % 2 == 0:
                                nc.scalar.copy(osb[:, c, ns], po[:, :])
                            else:
                                nc.vector.tensor_copy(osb[:, c, ns], po[:, :])
                    if mp >= RS // 512 - 2:
                        # split the last two stores across both queues so the
                        # final write does not trail the compute
                        for hh in range(2):
                            [nc.sync, nc.scalar][hh].dma_start(
                                out=out_d[
                                    mp * 512 + hh * 256:mp * 512 + (hh + 1) * 256, :
                                ].rearrange("(c p) m -> p c m", p=128),
                                in_=osb[:, 2 * hh:2 * (hh + 1), :],
                            )
                    else:
                        [nc.sync, nc.scalar][mp % 2].dma_start(
                            out=out_d[mp * 512:(mp + 1) * 512, :].rearrange(
                                "(c p) m -> p c m", p=128
                            ),
                            in_=osb[:, :, :],
                        )

    nc.compile()
    return nc


def _prepare(inputs):
    x = np.ascontiguousarray(np.asarray(inputs["x"], dtype=np.float32))
    fc1_w = np.asarray(inputs["fc1_w"], dtype=np.float32)
    fc2_w = np.asarray(inputs["fc2_w"], dtype=np.float32)
    fc2_b = np.asarray(inputs["fc2_b"], dtype=np.float32)
    bf = np.asarray(inputs["bf_params"], dtype=np.float32)

    import ml_dtypes

    bf16 = ml_dtypes.bfloat16
    Bm = _butterfly_matrix(bf)
    BT = np.ascontiguousarray(Bm.T).astype(np.float32)  # lhsT for w = B @ h
    # only the two diagonal 128x128 blocks matter (B is block-diagonal)
    BT2 = np.concatenate(
        [BT[0:128, 0:128], BT[128:256, 128:256]], axis=1
    ).astype(bf16)  # [128, 256]

    w1T = fc1_w.T.astype(bf16)  # [784, 256]
    # p-major pack: w1p[p, k, m] = w1T[k*128+p, m]
    w1p = np.ascontiguousarray(
        w1T[: KC6 * 128].reshape(KC6, 128, HID).transpose(1, 0, 2)
    ).reshape(128, KC6 * HID)
    w1tl = np.ascontiguousarray(w1T[KC6 * 128:])  # [16, 256]

    w2T = fc2_w.T.astype(bf16)  # [256, 1000]
    w2p = np.ascontiguousarray(
        w2T.reshape(2, 128, OUT_DIM).transpose(1, 0, 2)
    ).reshape(128, 2 * OUT_DIM)

    smalls = np.zeros((8, 128), dtype=np.float32)
    smalls[0] = inputs["bn1_gamma"][0:128]
    smalls[1] = inputs["bn1_gamma"][128:256]
    smalls[2] = inputs["bn1_beta"][0:128]
    smalls[3] = inputs["bn1_beta"][128:256]
    smalls[4] = inputs["bn2_gamma"][0:128]
    smalls[5] = inputs["bn2_gamma"][128:256]
    smalls[6] = inputs["bn2_beta"][0:128]
    smalls[7] = inputs["bn2_beta"][128:256]

    has_bias = bool(np.any(fc2_b != 0))

    in_maps = []
    for i in range(NCORES):
        xs = x[i * RS:(i + 1) * RS].astype(bf16)  # [4096, 784] rows x feat
        # xT[f, r] = xs[r, f]; pack xp[rc, p, k, m] = xT[k*128+p, rc*512+m]
        # = xs[rc*512+m, k*128+p]
        x6 = xs[:, : KC6 * 128].reshape(RC, RCW, KC6, 128)
        xp = np.ascontiguousarray(x6.transpose(0, 3, 2, 1)).reshape(
            RC, 128, KC6 * RCW
        )
        # tail: xtl[p, rc, m] = xs[rc*512+m, 768+p]
        xtl = np.ascontiguousarray(
            xs[:, KC6 * 128:].reshape(RC, RCW, KREM).transpose(2, 0, 1)
        ).reshape(KREM, RC * RCW)
        m = {
            "xT": xp,
            "xtl": xtl,
            "w1T": w1p,
            "w1tl": w1tl,
            "bT": BT2,
            "w2T": w2p,
            "smalls": smalls,
        }
        if has_bias:
            m["b2row"] = np.ascontiguousarray(fc2_b.reshape(1, OUT_DIM))
        in_maps.append(m)
    return in_maps, has_bias


def run(inputs, trace=False, trace_kwargs=None):
    from concourse.bass_utils import run_bass_kernel_spmd

    in_maps, has_bias = _prepare(inputs)
    key = ("prog", has_bias, RECOMPUTE_FC1)
    if key not in _cache:
        _cache[key] = _build(has_bias, RECOMPUTE_FC1)
    nc = _cache[key]

    kw = {}
    if trace:
        kw["trace"] = True
        if trace_kwargs:
            kw["trace_kwargs"] = trace_kwargs
    res = run_bass_kernel_spmd(nc, in_maps, core_ids=list(range(NCORES)), **kw)
    out = np.concatenate(
        [res.results[i]["out"].astype(np.float32) for i in range(NCORES)], axis=0
    )
    return out, res


def kernel(**inputs):
    out, _ = run(inputs, trace=False)
    return out


# revision 24
# speedup vs baseline: 1.0164x; 1.0164x over previous
"""Trainium2 Bass kernel for nn_HBClassicNet.

Net: fc1 -> BN1(+ReLU) -> poincare log-map -> 3-stage butterfly -> exp-map
     -> BN2(+ReLU) -> fc2

Key algebraic simplifications (host-side, batch-independent only):
  * The 3 butterfly stages compose into one 256x256 block-diagonal matrix B
    (two independent 128x128 blocks -> only those blocks are shipped).
  * The log/exp-map scales are per-row scalars and commute with B.
  * fc1 bias cancels exactly in BN1 (affine followed by batch-norm).

Sharding: pure data-parallel over the batch (32768 rows -> 8 x 4096).
BN batch statistics are made global with two tiny AllGathers (one ring
sweep instead of AllReduce's two) + a local 3-op tree sum.  A dummy
AllGather issued first-thing absorbs the multi-10us core launch stagger
and CC firmware warmup concurrently with the x load and fc1, so the real
stats collectives only pay ~1us trigger skew + ~6-13us exec.

Performance notes (from perfetto/NTFF analysis):
  * PE runs ~1.2-1.45GHz here; fc1 (57K cols) / fc2 (64K cols) dominate.
  * bf16 everywhere the ~2e-2 error budget allows.  fp8 is NOT usable:
    fc2 has ~250x output cancellation (max|out| ~ 0.004 from O(1) terms).
  * Inputs are repacked on the host so every DMA reads contiguous
    per-partition lines (6KB), and descriptors are spread across the two
    hardware DGE queues (sync/scalar).  gpsimd.dma_start (software DGE)
    is avoided for large patterns - the Q7 descriptor generation is slow.
  * Row norms accumulate into one [2, RCW] PSUM tile (matmul with
    [ones|0] / [0|ones] stationary vectors, interleaved accumulation
    groups); a single copy moves both norms out.
  * The T-coefficient chain is computed per half-shard with ACT ops
    grouped by function (Sqrt,Sqrt,Ln,Ln,Ln,Ln,Tanh,Tanh) - each ACT
    table switch costs 1.28us.
  * T is broadcast across partitions by gpsimd.partition_broadcast from
    a dedicated tile pool (sharing a pool with DVE-recycled tiles
    serializes the Pool queue against DVE).
  * BN2+fc2 are interleaved per row-chunk; output is written bf16 and
    upcast to f32 on the host.
"""

import numpy as np

B_FULL, IN_DIM, HID, OUT_DIM = 32768, 784, 256, 1000
NCORES = 8
RS = B_FULL // NCORES  # 4096 rows per shard
L, CURV = 3, 1e-3
LOG2_H = 8
EPS_BN = 1e-5

RC = 8          # row chunks per shard
RCW = RS // RC  # 512 rows per chunk
KC6 = 6         # full 128-partition K chunks of IN_DIM
KREM = IN_DIM - KC6 * 128  # 16

RECOMPUTE_FC1 = False

_cache = {}


def _butterfly_matrix(params):
    """Compose the L butterfly stages into one dense [HID, HID] matrix (f64)."""
    p64 = np.asarray(params, dtype=np.float64)
    Bm = np.eye(HID, dtype=np.float64)
    off = 0
    for l in range(L):
        bs = 1 << (l % LOG2_H)
        nb = HID // (2 * bs)
        a = p64[off:off + nb]
        b = p64[off + nb:off + 2 * nb]
        S = np.zeros((HID, HID), dtype=np.float64)
        for blk in range(nb):
            base = blk * 2 * bs
            i1 = np.arange(base, base + bs)
            i2 = i1 + bs
            S[i1, i1] = a[blk]
            S[i1, i2] = b[blk]
            S[i2, i1] = -b[blk]
            S[i2, i2] = a[blk]
        Bm = S @ Bm
        off += 2 * nb
    return Bm


def _build(has_bias, recompute=RECOMPUTE_FC1):
    import concourse.bacc as bacc
    import concourse.tile as tile
    import concourse.mybir as mybir

    f32 = mybir.dt.float32
    bf16 = mybir.dt.bfloat16
    AF = mybir.ActivationFunctionType
    ALU = mybir.AluOpType

    nc = bacc.Bacc(
        "TRN2",
        target_bir_lowering=False,
        debug=False,
        enable_asserts=False,
        num_devices=NCORES,
    )

    # p-major packed layouts: per-partition data is contiguous in DRAM so
    # DMA packets are large (6KB lines) instead of 1KB strided
    xT_d = nc.dram_tensor("xT", [RC, 128, KC6 * RCW], bf16, kind="ExternalInput")
    xtl_d = nc.dram_tensor("xtl", [KREM, RC * RCW], bf16, kind="ExternalInput")
    w1tl_d = nc.dram_tensor("w1tl", [KREM, HID], bf16, kind="ExternalInput")
    w1T_d = nc.dram_tensor("w1T", [128, KC6 * HID], bf16, kind="ExternalInput")
    bT_d = nc.dram_tensor("bT", [128, 2 * 128], bf16, kind="ExternalInput")
    w2T_d = nc.dram_tensor("w2T", [128, 2 * OUT_DIM], bf16, kind="ExternalInput")
    smalls_d = nc.dram_tensor("smalls", [8, 128], f32, kind="ExternalInput")
    if has_bias:
        b2_d = nc.dram_tensor("b2row", [1, OUT_DIM], f32, kind="ExternalInput")
    out_d = nc.dram_tensor("out", [RS, OUT_DIM], bf16, kind="ExternalOutput")

    with tile.TileContext(nc) as tc:
        with (
            tc.tile_pool(name="const", bufs=1) as constp,
            tc.tile_pool(name="big", bufs=1) as bigp,
            tc.tile_pool(name="row", bufs=1) as rowp,
            tc.tile_pool(name="small", bufs=1) as smallp,
            tc.tile_pool(name="cmp", bufs=16) as cmpp,
            tc.tile_pool(name="sqt", bufs=6) as sqtp,
            tc.tile_pool(name="tbp", bufs=3) as tbp,
            tc.tile_pool(name="outp", bufs=3) as outp,
            tc.tile_pool(name="psmm", bufs=6, space="PSUM") as psmm,
            tc.tile_pool(name="psnorm", bufs=2, space="PSUM") as psnorm,
            tc.tile_pool(name="dram", bufs=1, space="DRAM") as dramp,
        ):
            # warmup collective first: absorbs core launch stagger + CC
            # firmware warmup concurrently with the x load and fc1
            with nc.named_scope("warmup"):
                wuin = dramp.tile([128, 1], f32, tag="wuin")
                wuout = dramp.tile([NCORES * 128, 1], f32, tag="wuout")
                wusb = smallp.tile([128, 1], f32, tag="wusb")
                nc.vector.memset(wusb[:, :], 0.0)
                nc.scalar.dma_start(out=wuin[:, :], in_=wusb[:, :])
                nc.gpsimd.collective_compute(
                    "AllGather",
                    ALU.bypass,
                    replica_groups=[list(range(NCORES))],
                    ins=[wuin.opt()],
                    outs=[wuout.opt()],
                )

            # ---------------- constants (weights first, then x chunk 0) ------
            w1t6 = constp.tile([128, KC6, HID], bf16, tag="w1t6")
            nc.sync.dma_start(
                out=w1t6[:, :, :],
                in_=w1T_d[:, :].rearrange("p (k m) -> p k m", k=KC6),
            )
            w1t1 = constp.tile([KREM, HID], bf16, tag="w1t1")
            nc.scalar.dma_start(out=w1t1[:, :], in_=w1tl_d[:, :])

            xall = bigp.tile([128, RC, KC6, RCW], bf16, tag="xall")
            xtail = bigp.tile([KREM, RC, RCW], bf16, tag="xtail")

            def xload(rc):
                engs = [nc.sync, nc.scalar]
                if rc == 0:
                    # k=0 alone first: the first fc1 matmul only waits ~130KB
                    nc.sync.dma_start(
                        out=xall[:, 0, 0:1, :],
                        in_=xT_d[0, :, 0:RCW].rearrange("p (k m) -> p k m", k=1),
                    )
                    nc.scalar.dma_start(
                        out=xall[:, 0, 1:3, :],
                        in_=xT_d[0, :, RCW:3 * RCW].rearrange(
                            "p (k m) -> p k m", k=2
                        ),
                    )
                    nc.sync.dma_start(
                        out=xall[:, 0, 3:6, :],
                        in_=xT_d[0, :, 3 * RCW:6 * RCW].rearrange(
                            "p (k m) -> p k m", k=3
                        ),
                    )
                    nc.scalar.dma_start(
                        out=xtail[:, :, :],
                        in_=xtl_d[:, :].rearrange("p (r m) -> p r m", r=RC),
                    )
                    return
                KH = KC6 // 2
                for h in range(2):
                    ks = slice(h * KH * RCW, (h + 1) * KH * RCW)
                    engs[h % 2].dma_start(
                        out=xall[:, rc, h * KH:(h + 1) * KH, :],
                        in_=xT_d[rc, :, ks].rearrange("p (k m) -> p k m", k=KH),
                    )

            with nc.named_scope("xload"):
                xload(0)

            bt_sb = constp.tile([128, 2, 128], bf16, tag="bt")
            nc.gpsimd.dma_start(
                out=bt_sb[:, :, :],
                in_=bT_d[:, :].rearrange("p (c m) -> p c m", c=2),
            )
            w2t_sb = constp.tile([128, 2, OUT_DIM], bf16, tag="w2t")
            nc.gpsimd.dma_start(
                out=w2t_sb[:, :, :],
                in_=w2T_d[:, :].rearrange("p (k m) -> p k m", k=2),
            )
            smalls = constp.tile([128, 8], f32, tag="smalls")
            nc.gpsimd.dma_start(
                out=smalls[:, :], in_=smalls_d[:, :].rearrange("c p -> p c")
            )
            if has_bias:
                b2row = constp.tile([1, OUT_DIM], bf16, tag="b2row")
                b2f = constp.tile([1, OUT_DIM], f32, tag="b2f")
                nc.gpsimd.dma_start(out=b2f[:, :], in_=b2_d[:, :])
                nc.scalar.copy(b2row[:, :], b2f[:, :])

            ones_k = constp.tile([128, 1], bf16, tag="ones_k")
            nc.vector.memset(ones_k[:, :], 1.0)
            nsel = constp.tile([128, 2, 2], bf16, tag="nsel")
            nc.vector.memset(nsel[:, :, :], 0.0)
            nc.vector.memset(nsel[:, 0, 0:1], 1.0)
            nc.vector.memset(nsel[:, 1, 1:2], 1.0)
            ones_m = constp.tile([1, 128], bf16, tag="ones_m")
            nc.vector.memset(ones_m[:, :], 1.0)
            eps_t = constp.tile([128, 1], f32, tag="eps_t")
            nc.vector.memset(eps_t[:, :], float(EPS_BN))

            with nc.named_scope("xload"):
                for rc in range(1, RC):
                    xload(rc)

            # ---------------- fc1 ----------------
            stat1 = smallp.tile([128, 2, RC, 6], f32, tag="stat1")
            if not recompute:
                h = [
                    bigp.tile([128, RS], bf16, tag=f"h{m}", name=f"h{m}")
                    for m in range(2)
                ]

            def fc1_chunk(rc, mc):
                ms = slice(mc * 128, (mc + 1) * 128)
                ph = psmm.tile([128, RCW], f32, tag="psmm", name="ph")
                for k in range(KC6):
                    nc.tensor.matmul(
                        ph[:, :],
                        w1t6[:, k, ms],
                        xall[:, rc, k, :],
                        start=(k == 0),
                        stop=False,
                    )
                nc.tensor.matmul(
                    ph[:, :], w1t1[:, ms], xtail[:, rc, :],
                    start=False, stop=True,
                )
                return ph

            with nc.named_scope("fc1"):
                for rc in range(RC):
                    cs = slice(rc * RCW, (rc + 1) * RCW)
                    for mc in range(2):
                        ph = fc1_chunk(rc, mc)
                        nc.vector.bn_stats(stat1[:, mc, rc, :], ph[:, :])
                        if not recompute:
                            if (rc + mc) % 2 == 0:
                                nc.scalar.copy(h[mc][:, cs], ph[:, :])
                            else:
                                nc.vector.tensor_copy(h[mc][:, cs], ph[:, :])

            # ---------------- stats AllGather + local sum ----------------
            def stats_allgather(stat, tag):
                """stat [128,2,RC,6] -> allr [128, 4]: global (sum mean, sum E[x^2])."""
                aggr = smallp.tile([128, 2, 2], f32, tag=f"aggr{tag}", name=f"aggr{tag}")
                pay = smallp.tile([128, 4], f32, tag=f"pay{tag}", name=f"pay{tag}")
                for mc in range(2):
                    nc.vector.bn_aggr(aggr[:, mc, :], stat[:, mc, :, :])
                msq = cmpp.tile([128, 2], f32, tag="cmp", name=f"msq{tag}")
                nc.vector.tensor_mul(msq[:, :], aggr[:, :, 0], aggr[:, :, 0])
                nc.vector.tensor_copy(pay[:, 0:2], aggr[:, :, 0])
                nc.vector.tensor_add(pay[:, 2:4], aggr[:, :, 1], msq[:, :])
                arin = dramp.tile([128, 4], f32, tag=f"arin{tag}", name=f"arin{tag}")
                arout = dramp.tile(
                    [NCORES * 128, 4], f32, tag=f"arout{tag}", name=f"arout{tag}"
                )
                nc.sync.dma_start(out=arin[:, :], in_=pay[:, :])
                nc.gpsimd.collective_compute(
                    "AllGather",
                    ALU.bypass,
                    replica_groups=[list(range(NCORES))],
                    ins=[arin.opt()],
                    outs=[arout.opt()],
                )
                allg = smallp.tile(
                    [128, NCORES, 4], f32, tag=f"allg{tag}", name=f"allg{tag}"
                )
                nc.sync.dma_start(
                    out=allg[:, :, :],
                    in_=arout[:, :].rearrange("(r p) c -> p r c", p=128),
                )
                s4 = cmpp.tile([128, 4, 4], f32, tag="cmp", name=f"s4{tag}")
                nc.vector.tensor_add(s4[:, :, :], allg[:, 0:4, :], allg[:, 4:8, :])
                s2 = cmpp.tile([128, 2, 4], f32, tag="cmp", name=f"s2{tag}")
                nc.vector.tensor_add(s2[:, :, :], s4[:, 0:2, :], s4[:, 2:4, :])
                allr = smallp.tile([128, 1, 4], f32, tag=f"allr{tag}", name=f"allr{tag}")
                nc.vector.tensor_add(allr[:, :, :], s2[:, 0:1, :], s2[:, 1:2, :])
                return allr

            def bn_scale_bias(allr, gcol, bcol, tag):
                """Global stats -> per-partition scale/bias tiles [128, 2]."""
                scale = smallp.tile([128, 2], f32, tag=f"scale{tag}", name=f"scale{tag}")
                bias = smallp.tile([128, 2], f32, tag=f"bias{tag}", name=f"bias{tag}")
                mean = cmpp.tile([128, 2], f32, tag="cmp", name=f"mean{tag}")
                nc.vector.tensor_scalar_mul(mean[:, :], allr[:, 0, 0:2], 1.0 / NCORES)
                m2 = cmpp.tile([128, 2], f32, tag="cmp", name=f"m2{tag}")
                nc.vector.tensor_mul(m2[:, :], mean[:, :], mean[:, :])
                var = cmpp.tile([128, 2], f32, tag="cmp", name=f"var{tag}")
                nc.vector.scalar_tensor_tensor(
                    out=var[:, :], in0=allr[:, 0, 2:4], scalar=1.0 / NCORES,
                    in1=m2[:, :], op0=ALU.mult, op1=ALU.subtract,
                )
                std = cmpp.tile([128, 2], f32, tag="cmp", name=f"std{tag}")
                nc.scalar.activation(std[:, :], var[:, :], AF.Sqrt, bias=eps_t[:, :])
                rstd = cmpp.tile([128, 2], f32, tag="cmp", name=f"rstd{tag}")
                nc.vector.reciprocal(rstd[:, :], std[:, :])
                nc.vector.tensor_mul(scale[:, :], rstd[:, :], smalls[:, gcol:gcol + 2])
                mneg = cmpp.tile([128, 2], f32, tag="cmp", name=f"mneg{tag}")
                nc.vector.scalar_tensor_tensor(
                    out=mneg[:, :], in0=allr[:, 0, 0:2], scalar=-1.0 / NCORES,
                    in1=scale[:, :], op0=ALU.mult, op1=ALU.mult,
                )
                nc.vector.tensor_add(bias[:, :], mneg[:, :], smalls[:, bcol:bcol + 2])
                return scale, bias

            with nc.named_scope("ar1"):
                allr1 = stats_allgather(stat1, "1")
                scale1, bias1 = bn_scale_bias(allr1, 0, 2, "1")

            # ---------------- middle: BN1+ReLU, butterfly, norms, T ----------
            hbn = [
                bigp.tile([128, RS], bf16, tag=f"hbn{m}", name=f"hbn{m}")
                for m in range(2)
            ]
            w = [
                bigp.tile([128, RS], bf16, tag=f"w{m}", name=f"w{m}")
                for m in range(2)
            ]
            ht = [
                bigp.tile([128, RS], bf16, tag=f"ht{m}", name=f"ht{m}")
                for m in range(2)
            ]
            trow = rowp.tile([1, RS], bf16, tag="trow")
            stat2 = smallp.tile([128, 2, RC, 6], f32, tag="stat2")
            nrow2 = rowp.tile([2, RS], f32, tag="nrow2")

            nhalf = RC // 2
            HW = nhalf * RCW  # 2048 rows per half

            def mid_chunk(rc):
                cs = slice(rc * RCW, (rc + 1) * RCW)
                with nc.named_scope("bn1"):
                    for mc in range(2):
                        if recompute:
                            src_ = fc1_chunk(rc, mc)[:, :]
                        else:
                            src_ = h[mc][:, cs]
                        nc.scalar.activation(
                            hbn[mc][:, cs], src_, AF.Relu,
                            bias=bias1[:, mc:mc + 1], scale=scale1[:, mc:mc + 1],
                        )
                with nc.named_scope("norms"):
                    pn = psnorm.tile([2, RCW], f32, tag="psn", name="pn")
                    sqs = []
                    for mc in range(2):
                        sqt = sqtp.tile([128, RCW], bf16, tag="sqt", name="sqt")
                        nc.vector.tensor_mul(sqt[:, :], hbn[mc][:, cs], hbn[mc][:, cs])
                        sqs.append(sqt)
                    nc.tensor.matmul(
                        pn[:, :], nsel[:, 0, :], sqs[0][:, :],
                        start=True, stop=False, skip_group_check=True,
                    )
                    nc.tensor.matmul(
                        pn[:, :], nsel[:, 0, :], sqs[1][:, :],
                        start=False, stop=False, skip_group_check=True,
                    )
                    for mc in range(2):
                        pw = psmm.tile([128, RCW], f32, tag="psmm", name="pw")
                        nc.tensor.matmul(
                            pw[:, :], bt_sb[:, mc, :], hbn[mc][:, cs],
                            start=True, stop=True,
                        )
                        if mc == 0:
                            nc.scalar.copy(w[mc][:, cs], pw[:, :])
                        else:
                            nc.vector.tensor_copy(w[mc][:, cs], pw[:, :])
                        sqwt = sqtp.tile([128, RCW], bf16, tag="sqt", name="sqwt")
                        nc.vector.tensor_mul(sqwt[:, :], w[mc][:, cs], w[mc][:, cs])
                        nc.tensor.matmul(
                            pn[:, :], nsel[:, 1, :], sqwt[:, :],
                            start=False, stop=(mc == 1), skip_group_check=True,
                        )
                    nc.scalar.copy(nrow2[:, cs], pn[:, :])

            def rowscalars_half(hf):
                C = HW // 128  # 16
                hs = slice(hf * HW, (hf + 1) * HW)
                ncc = cmpp.tile([128, 2, C], f32, tag="cmp", name=f"ncc{hf}")
                nc.scalar.dma_start(
                    out=ncc[:, 0, :],
                    in_=nrow2[0:1, hs].rearrange("o (a b) -> o a b", a=128),
                )
                nc.scalar.dma_start(
                    out=ncc[:, 1, :],
                    in_=nrow2[1:2, hs].rearrange("o (a b) -> o a b", a=128),
                )
                sq2 = cmpp.tile([128, 2, C], f32, tag="cmp", name=f"sq2{hf}")
                nc.scalar.activation(
                    sq2[:, :, :], ncc[:, :, :], AF.Sqrt, scale=float(CURV)
                )
                sn1 = cmpp.tile([128, C], f32, tag="cmp", name=f"sn1{hf}")
                nc.vector.tensor_scalar(
                    out=sn1[:, :], in0=sq2[:, 0, :],
                    scalar1=1.0 - 1e-6, scalar2=1e-7,
                    op0=ALU.min, op1=ALU.max,
                )
                snwf = cmpp.tile([128, C], f32, tag="cmp", name=f"snwf{hf}")
                nc.vector.tensor_scalar(
                    out=snwf[:, :], in0=sq2[:, 1, :],
                    scalar1=1e-20, scalar2=None, op0=ALU.max,
                )
                la = cmpp.tile([128, C], f32, tag="cmp", name=f"la{hf}")
                nc.scalar.activation(la[:, :], sn1[:, :], AF.Ln, bias=1.0, scale=1.0)
                lb = cmpp.tile([128, C], f32, tag="cmp", name=f"lb{hf}")
                nc.scalar.activation(lb[:, :], sn1[:, :], AF.Ln, bias=1.0, scale=-1.0)
                at2 = cmpp.tile([128, C], f32, tag="cmp", name=f"at2{hf}")
                nc.vector.tensor_sub(at2[:, :], la[:, :], lb[:, :])  # 2*artanh
                r1 = cmpp.tile([128, C], f32, tag="cmp", name=f"r1{hf}")
                nc.vector.reciprocal(r1[:, :], sn1[:, :])
                m1 = cmpp.tile([128, C], f32, tag="cmp", name=f"m1{hf}")
                nc.vector.tensor_mul(m1[:, :], at2[:, :], r1[:, :])  # 2*ls
                m2t = cmpp.tile([128, C], f32, tag="cmp", name=f"m2t{hf}")
                nc.vector.tensor_mul(m2t[:, :], m1[:, :], snwf[:, :])  # 2*sn2
                th = cmpp.tile([128, C], f32, tag="cmp", name=f"th{hf}")
                nc.scalar.activation(th[:, :], m2t[:, :], AF.Tanh, scale=0.5)
                rw = cmpp.tile([128, C], f32, tag="cmp", name=f"rw{hf}")
                nc.vector.reciprocal(rw[:, :], snwf[:, :])
                tco = cmpp.tile([128, C], bf16, tag="cmp", name=f"tco{hf}")
                nc.vector.tensor_mul(tco[:, :], th[:, :], rw[:, :])
                nc.scalar.dma_start(
                    out=trow[0:1, hs].rearrange("o (a b) -> o a b", a=128),
                    in_=tco[:, :],
                )

            def applyT_chunk(rc):
                cs = slice(rc * RCW, (rc + 1) * RCW)
                tb = tbp.tile([128, RCW], bf16, tag="tb", name="tb")
                nc.gpsimd.partition_broadcast(tb[:, :], trow[0:1, cs])
                for mc in range(2):
                    nc.vector.tensor_mul(ht[mc][:, cs], w[mc][:, cs], tb[:, :])
                    nc.vector.bn_stats(stat2[:, mc, rc, :], ht[mc][:, cs])

            with nc.named_scope("middle"):
                for rc in range(nhalf):
                    mid_chunk(rc)
                rowscalars_half(0)
                for rc in range(nhalf, RC):
                    mid_chunk(rc)
                for rc in range(nhalf):
                    applyT_chunk(rc)
                rowscalars_half(1)
                for rc in range(nhalf, RC):
                    applyT_chunk(rc)

            with nc.named_scope("ar2"):
                allr2 = stats_allgather(stat2, "2")
                scale2, bias2 = bn_scale_bias(allr2, 4, 6, "2")

            # ---------------- BN2 + fc2 interleaved + store ----------------
            ht2 = [
                bigp.tile([128, RS], bf16, tag=f"ht2{m}", name=f"ht2{m}")
                for m in range(2)
            ]
            NH = OUT_DIM // 2  # 500
            with nc.named_scope("fc2"):
                for mp in range(RS // 512):  # 8 quad-row-chunks
                    rc = mp
                    cs = slice(rc * RCW, (rc + 1) * RCW)
                    with nc.named_scope("bn2"):
                        for mc in range(2):
                            nc.scalar.activation(
                                ht2[mc][:, cs], ht[mc][:, cs], AF.Relu,
                                bias=bias2[:, mc:mc + 1],
                                scale=scale2[:, mc:mc + 1],
                            )
                    osb = outp.tile([128, 4, OUT_DIM], bf16, tag="osb", name="osb")
                    for c in range(4):
                        m = 4 * mp + c
                        rs_ = slice(m * 128, (m + 1) * 128)
                        for nch in range(2):
                            ns = slice(nch * NH, (nch + 1) * NH)
                            po = psmm.tile([128, NH], f32, tag="psmm", name="po")
                            nc.tensor.matmul(
                                po[:, :], ht2[0][:, rs_], w2t_sb[:, 0, ns],
                                start=True, stop=False,
                            )
                            nc.tensor.matmul(
                                po[:, :], ht2[1][:, rs_], w2t_sb[:, 1, ns],
                                start=False, stop=(not has_bias),
                            )
                            if has_bias:
                                nc.tensor.matmul(
                                    po[:, :], ones_m[:, :], b2row[0:1, ns],
                                    start=False, stop=True,
                                )
                            if (m + nch) % 2 == 0:
                                nc.scalar.copy(osb[:, c, ns], po[:, :])
                            else:
                                nc.vector.tensor_copy(osb[:, c, ns], po[:, :])
                    if mp >= RS // 512 - 2:
                        # split the last two stores across both queues so the
                        # final write does not trail the compute
                        for hh in range(2):
                            [nc.sync, nc.scalar][hh].dma_start(
                                out=out_d[
                                    mp * 512 + hh * 256:mp * 512 + (hh + 1) * 256, :
                                ].rearrange("(c p) m -> p c m", p=128),
                                in_=osb[:, 2 * hh:2 * (hh + 1), :],
                            )
                    else:
                        [nc.sync, nc.scalar][mp % 2].dma_start(
                            out=out_d[mp * 512:(mp + 1) * 512, :].rearrange(
                                "(c p) m -> p c m", p=128
                            ),
                            in_=osb[:, :, :],
                        )

    nc.compile()
    return nc


def _prepare(inputs):
    x = np.ascontiguousarray(np.asarray(inputs["x"], dtype=np.float32))
    fc1_w = np.asarray(inputs["fc1_w"], dtype=np.float32)
    fc2_w = np.asarray(inputs["fc2_w"], dtype=np.float32)
    fc2_b = np.asarray(inputs["fc2_b"], dtype=np.float32)
    bf = np.asarray(inputs["bf_params"], dtype=np.float32)

    import ml_dtypes

    bf16 = ml_dtypes.bfloat16
    Bm = _butterfly_matrix(bf)
    BT = np.ascontiguousarray(Bm.T).astype(np.float32)  # lhsT for w = B @ h
    # only the two diagonal 128x128 blocks matter (B is block-diagonal)
    BT2 = np.concatenate(
        [BT[0:128, 0:128], BT[128:256, 128:256]], axis=1
    ).astype(bf16)  # [128, 256]

    w1T = fc1_w.T.astype(bf16)  # [784, 256]
    # p-major pack: w1p[p, k, m] = w1T[k*128+p, m]
    w1p = np.ascontiguousarray(
        w1T[: KC6 * 128].reshape(KC6, 128, HID).transpose(1, 0, 2)
    ).reshape(128, KC6 * HID)
    w1tl = np.ascontiguousarray(w1T[KC6 * 128:])  # [16, 256]

    w2T = fc2_w.T.astype(bf16)  # [256, 1000]
    w2p = np.ascontiguousarray(
        w2T.reshape(2, 128, OUT_DIM).transpose(1, 0, 2)
    ).reshape(128, 2 * OUT_DIM)

    smalls = np.zeros((8, 128), dtype=np.float32)
    smalls[0] = inputs["bn1_gamma"][0:128]
    smalls[1] = inputs["bn1_gamma"][128:256]
    smalls[2] = inputs["bn1_beta"][0:128]
    smalls[3] = inputs["bn1_beta"][128:256]
    smalls[4] = inputs["bn2_gamma"][0:128]
    smalls[5] = inputs["bn2_gamma"][128:256]
    smalls[6] = inputs["bn2_beta"][0:128]
    smalls[7] = inputs["bn2_beta"][128:256]

    has_bias = bool(np.any(fc2_b != 0))

    in_maps = []
    for i in range(NCORES):
        xs = x[i * RS:(i + 1) * RS].astype(bf16)  # [4096, 784] rows x feat
        # xT[f, r] = xs[r, f]; pack xp[rc, p, k, m] = xT[k*128+p, rc*512+m]
        # = xs[rc*512+m, k*128+p]
        x6 = xs[:, : KC6 * 128].reshape(RC, RCW, KC6, 128)
        xp = np.ascontiguousarray(x6.transpose(0, 3, 2, 1)).reshape(
            RC, 128, KC6 * RCW
        )
        # tail: xtl[p, rc, m] = xs[rc*512+m, 768+p]
        xtl = np.ascontiguousarray(
            xs[:, KC6 * 128:].reshape(RC, RCW, KREM).transpose(2, 0, 1)
        ).reshape(KREM, RC * RCW)
        m = {
            "xT": xp,
            "xtl": xtl,
            "w1T": w1p,
            "w1tl": w1tl,
            "bT": BT2,
            "w2T": w2p,
            "smalls": smalls,
        }
        if has_bias:
            m["b2row"] = np.ascontiguousarray(fc2_b.reshape(1, OUT_DIM))
        in_maps.append(m)
    return in_maps, has_bias


def run(inputs, trace=False, trace_kwargs=None):
    from concourse.bass_utils import run_bass_kernel_spmd

    in_maps, has_bias = _prepare(inputs)
    key = ("prog", has_bias, RECOMPUTE_FC1)
    if key not in _cache:
        _cache[key] = _build(has_bias, RECOMPUTE_FC1)
    nc = _cache[key]

    kw = {}
    if trace:
        kw["trace"] = True
        if trace_kwargs:
            kw["trace_kwargs"] = trace_kwargs
    res = run_bass_kernel_spmd(nc, in_maps, core_ids=list(range(NCORES)), **kw)
    out = np.concatenate(
        [res.results[i]["out"].astype(np.float32) for i in range(NCORES)], axis=0
    )
    return out, res


def kernel(**inputs):
    out, _ = run(inputs, trace=False)
    return out
